# revision 13
# baseline (speedup 1.0000x reference)
"""TextCNN-style conv layer (kernel sizes 3/4/5, EMB=300 -> DEPTH=256, bias,
ReLU, max-pool over time) as a Bass/Tile kernel for 8 Trainium2 NeuronCores.

Strategy: data-parallel over batch (8 samples per core), weights replicated.
Conv is computed as accumulated matmuls: for branch n, window output
y[d, i] = sum_{j<n, e<300} x[i+j, e] * W[d, j*300+e].  With x transposed
host-side to [EMB, SEQ] (EMB padded to 384 = 3*128), each (j, e-chunk) pair is
one K=128 matmul whose moving operand is just a free-dim slice x_t[chunk,
j:j+NW] -- no im2col materialization.  Weights are host-packed into
[128, 2, 36, 128] (k-in-chunk, depth-half, (branch,j,chunk), depth-in-half)
with zero padding for the ragged 300 % 128 = 44 chunk.  dtype float32r (FP22
multiplies at full PE rate, fp32 PSUM accumulate).

Epilogue: relu(max_i(y + b)) == max(0, max_i y + b) is computed as DVE
reduce_max over the window axis straight out of PSUM, then a broadcast bias
add + clamp at 0.  Output is staged [d, branch, half, sample] per core and
de-transposed on host.
"""

import numpy as np

B, SEQ, EMB = 64, 394, 300
DEPTH = 256
NCORES = 8
BPC = B // NCORES  # samples per core
EMB_PAD = 384  # 3 * 128
NS = (3, 4, 5)
NCOL = sum(3 * n for n in NS)  # 36 weight tiles per depth-half
COL_BASE = (0, 9, 21)  # column offset of each branch's (j, chunk) tiles

TRACE = False
LAST_RESULT = None

_built = None


def _build_bass():
    import concourse.mybir as mybir
    import concourse.tile as tile
    from concourse import bacc
    from contextlib import ExitStack

    f32 = mybir.dt.float32
    f32r = mybir.dt.float32r

    nc = bacc.Bacc("TRN2", target_bir_lowering=False)
    xt_d = nc.dram_tensor("xt", (BPC, 3, 128, SEQ), f32r, kind="ExternalInput")
    wp_d = nc.dram_tensor("wp", (128, 2, NCOL, 128), f32r, kind="ExternalInput")
    bp_d = nc.dram_tensor("bp", (128, 3, 2), f32, kind="ExternalInput")
    out_d = nc.dram_tensor("out_t", (128, 3, 2, BPC), f32, kind="ExternalOutput")

    with tile.TileContext(nc) as tc, ExitStack() as ctx:
        xpool = ctx.enter_context(tc.tile_pool(name="x", bufs=BPC))
        wpool = ctx.enter_context(tc.tile_pool(name="w", bufs=1))
        cpool = ctx.enter_context(tc.tile_pool(name="consts", bufs=1))
        spool = ctx.enter_context(tc.tile_pool(name="stage", bufs=1))
        pspool = ctx.enter_context(tc.tile_pool(name="ps", bufs=8, space="PSUM"))

        # Weights: one tile per (depth-half, branch), each filled by a single
        # DMA so consumers wait on exactly one DMA lane.
        wts = {}
        for dh in range(2):
            for br, n in enumerate(NS):
                sl = slice(COL_BASE[br], COL_BASE[br] + 3 * n)
                wt = wpool.tile([128, 3 * n, 128], f32r, tag=f"w{dh}{br}")
                nc.sync.dma_start(wt[:], wp_d[:, dh, sl, :])
                wts[dh, br] = wt

        bt = cpool.tile([128, 3, 2], f32)
        nc.sync.dma_start(bt[:], bp_d[:])

        xtiles = []
        for s in range(BPC):
            xtile = xpool.tile([128, 3, SEQ], f32r, tag="x")
            nc.sync.dma_start(xtile[:], xt_d[s].rearrange("c p t -> p c t"))
            xtiles.append(xtile)

        stage = spool.tile([128, 3, 2, BPC], f32)

        for dh in range(2):
            for br, n in enumerate(NS):
                nw = SEQ - n  # windows the reference maxes over
                # fp32r matmuls need an even moving-element count; window
                # nw (== SEQ-n, still a valid conv window) is computed but
                # excluded from the reduce.
                nmm = nw + (nw & 1)
                nj = 3 * n
                for s in range(BPC):
                    ps = pspool.tile([128, 512], f32, tag="ps")
                    for jc in range(nj):
                        j, c = divmod(jc, 3)
                        nc.tensor.matmul(
                            ps[:, :nmm],
                            lhsT=wts[dh, br][:, jc, :],
                            rhs=xtiles[s][:, c, j : j + nmm],
                            start=(jc == 0),
                            stop=(jc == nj - 1),
                        )
                    nc.vector.reduce_max(
                        stage[:, br, dh, s : s + 1],
                        ps[:, :nw],
                        axis=mybir.AxisListType.X,
                    )

        stage2 = spool.tile([128, 3, 2, BPC], f32)
        nc.vector.tensor_tensor(
            stage2[:],
            stage[:],
            bt[:, :, :, None].to_broadcast((128, 3, 2, BPC)),
            mybir.AluOpType.add,
        )
        nc.vector.tensor_scalar_max(stage2[:], stage2[:], 0.0)
        nc.sync.dma_start(out_d[:], stage2[:])

    nc.compile()
    return nc


def _pack_inputs(input, W1, W2, W3, b1, b2, b3):
    # x transposed to [B, EMB_PAD, SEQ], zero-padded in EMB
    xt = np.zeros((B, EMB_PAD, SEQ), np.float32)
    xt[:, :EMB, :] = np.asarray(input, np.float32).transpose(0, 2, 1)
    xt = xt.reshape(B, 3, 128, SEQ)

    wp = np.zeros((128, 2, NCOL, 128), np.float32)
    for br, (n, W) in enumerate(zip(NS, (W1, W2, W3))):
        W = np.asarray(W, np.float32)
        col = COL_BASE[br]
        for j in range(n):
            for c in range(3):
                k0 = j * EMB + c * 128
                kk = min(128, (j + 1) * EMB - k0)
                for dh in range(2):
                    wp[:kk, dh, col, :] = W[dh * 128 : (dh + 1) * 128, k0 : k0 + kk].T
                col += 1

    bp = np.empty((128, 3, 2), np.float32)
    for br, b in enumerate((b1, b2, b3)):
        b = np.asarray(b, np.float32).reshape(DEPTH)
        for dh in range(2):
            bp[:, br, dh] = b[dh * 128 : (dh + 1) * 128]
    return xt, wp, bp


def kernel(input, W1, W2, W3, b1, b2, b3):
    global _built, LAST_RESULT
    from concourse.bass_utils import run_bass_kernel_spmd

    xt, wp, bp = _pack_inputs(input, W1, W2, W3, b1, b2, b3)

    if _built is None:
        _built = _build_bass()
    nc = _built

    in_maps = [
        {"xt": xt[c * BPC : (c + 1) * BPC], "wp": wp, "bp": bp}
        for c in range(NCORES)
    ]
    res = run_bass_kernel_spmd(
        nc, in_maps, core_ids=list(range(NCORES)), trace=TRACE
    )
    LAST_RESULT = res

    out = np.empty((B, 3 * DEPTH), np.float32)
    for c in range(NCORES):
        arr = res.results[c]["out_t"]  # [128, 3, 2, BPC]
        out[c * BPC : (c + 1) * BPC] = arr.transpose(3, 1, 2, 0).reshape(BPC, 768)
    return out


# revision 14
# speedup vs baseline: 1.0139x; 1.0139x over previous
"""TextCNN-style conv layer (kernel sizes 3/4/5, EMB=300 -> DEPTH=256, bias,
ReLU, max-pool over time) as a Bass/Tile kernel for 8 Trainium2 NeuronCores.

Strategy: data-parallel over batch (8 samples per core), weights replicated.
Conv is computed as accumulated matmuls: for branch n, window output
y[d, i] = sum_{j<n, e<300} x[i+j, e] * W[d, j*300+e].  With x transposed
host-side to [EMB, SEQ] (EMB padded to 384 = 3*128), each (j, e-chunk) pair is
one K=128 matmul whose moving operand is just a free-dim slice x_t[chunk,
j:j+NW] -- no im2col materialization.  Weights are host-packed into
[128, 2, 36, 128] (k-in-chunk, depth-half, (branch,j,chunk), depth-in-half)
with zero padding for the ragged 300 % 128 = 44 chunk.  dtype float32r (FP22
multiplies at full PE rate, fp32 PSUM accumulate).

Epilogue: relu(max_i(y + b)) == max(0, max_i y + b) is computed as DVE
reduce_max over the window axis straight out of PSUM, then a broadcast bias
add + clamp at 0.  Output is staged [d, branch, half, sample] per core and
de-transposed on host.
"""

import numpy as np

B, SEQ, EMB = 64, 394, 300
DEPTH = 256
NCORES = 8
BPC = B // NCORES  # samples per core
EMB_PAD = 384  # 3 * 128
NS = (3, 4, 5)
NCOL = sum(3 * n for n in NS)  # 36 weight tiles per depth-half
COL_BASE = (0, 9, 21)  # column offset of each branch's (j, chunk) tiles

TRACE = False
LAST_RESULT = None

_built = None


def _build_bass():
    import concourse.mybir as mybir
    import concourse.tile as tile
    from concourse import bacc
    from contextlib import ExitStack

    f32 = mybir.dt.float32
    f32r = mybir.dt.float32r

    nc = bacc.Bacc("TRN2", target_bir_lowering=False)
    xt_d = nc.dram_tensor("xt", (BPC, 3, 128, SEQ), f32r, kind="ExternalInput")
    wp_d = nc.dram_tensor("wp", (128, 2, NCOL, 128), f32r, kind="ExternalInput")
    bp_d = nc.dram_tensor("bp", (128, 3, 2), f32, kind="ExternalInput")
    out_d = nc.dram_tensor("out_t", (128, 3, 2, BPC), f32, kind="ExternalOutput")

    with tile.TileContext(nc) as tc, ExitStack() as ctx:
        xpool = ctx.enter_context(tc.tile_pool(name="x", bufs=BPC))
        wpool = ctx.enter_context(tc.tile_pool(name="w", bufs=1))
        cpool = ctx.enter_context(tc.tile_pool(name="consts", bufs=1))
        spool = ctx.enter_context(tc.tile_pool(name="stage", bufs=1))
        pspool = ctx.enter_context(tc.tile_pool(name="ps", bufs=8, space="PSUM"))

        # Initial loads: HWDGE DMAs are FIFO per issuing engine (SP and ACT
        # rings), so interleave weight/x loads in first-use order and spread
        # them over both rings + the gpsimd SWDGE path; otherwise the first
        # matmul waits ~30us for every input to land.
        wts = {}
        xtiles = [None] * BPC

        def load_w(dh, br, eng):
            n = NS[br]
            sl = slice(COL_BASE[br], COL_BASE[br] + 3 * n)
            wt = wpool.tile([128, 3 * n, 128], f32r, tag=f"w{dh}{br}")
            eng.dma_start(wt[:], wp_d[:, dh, sl, :])
            wts[dh, br] = wt

        def load_x(s, eng):
            xtile = xpool.tile([128, 3, SEQ], f32r, tag="x")
            eng.dma_start(xtile[:], xt_d[s].rearrange("c p t -> p c t"))
            xtiles[s] = xtile

        load_w(0, 0, nc.sync)
        load_x(0, nc.scalar)
        load_x(1, nc.sync)
        load_x(2, nc.scalar)
        load_w(0, 1, nc.sync)
        load_x(3, nc.scalar)
        load_x(4, nc.sync)
        load_w(0, 2, nc.scalar)
        load_x(5, nc.sync)
        load_x(6, nc.scalar)
        load_x(7, nc.sync)
        load_w(1, 0, nc.scalar)
        load_w(1, 1, nc.sync)
        load_w(1, 2, nc.scalar)

        bt = cpool.tile([128, 3, 2], f32)
        nc.gpsimd.dma_start(bt[:], bp_d[:])

        stage = spool.tile([128, 3, 2, BPC], f32)

        for dh in range(2):
            for br, n in enumerate(NS):
                nw = SEQ - n  # windows the reference maxes over
                # fp32r matmuls need an even moving-element count; window
                # nw (== SEQ-n, still a valid conv window) is computed but
                # excluded from the reduce.
                nmm = nw + (nw & 1)
                nj = 3 * n
                for s in range(BPC):
                    ps = pspool.tile([128, 512], f32, tag="ps")
                    for jc in range(nj):
                        j, c = divmod(jc, 3)
                        nc.tensor.matmul(
                            ps[:, :nmm],
                            lhsT=wts[dh, br][:, jc, :],
                            rhs=xtiles[s][:, c, j : j + nmm],
                            start=(jc == 0),
                            stop=(jc == nj - 1),
                        )
                    nc.vector.reduce_max(
                        stage[:, br, dh, s : s + 1],
                        ps[:, :nw],
                        axis=mybir.AxisListType.X,
                    )

        stage2 = spool.tile([128, 3, 2, BPC], f32)
        nc.vector.tensor_tensor(
            stage2[:],
            stage[:],
            bt[:, :, :, None].to_broadcast((128, 3, 2, BPC)),
            mybir.AluOpType.add,
        )
        nc.vector.tensor_scalar_max(stage2[:], stage2[:], 0.0)
        nc.sync.dma_start(out_d[:], stage2[:])

    nc.compile()
    return nc


def _pack_inputs(input, W1, W2, W3, b1, b2, b3):
    # x transposed to [B, EMB_PAD, SEQ], zero-padded in EMB
    xt = np.zeros((B, EMB_PAD, SEQ), np.float32)
    xt[:, :EMB, :] = np.asarray(input, np.float32).transpose(0, 2, 1)
    xt = xt.reshape(B, 3, 128, SEQ)

    wp = np.zeros((128, 2, NCOL, 128), np.float32)
    for br, (n, W) in enumerate(zip(NS, (W1, W2, W3))):
        W = np.asarray(W, np.float32)
        col = COL_BASE[br]
        for j in range(n):
            for c in range(3):
                k0 = j * EMB + c * 128
                kk = min(128, (j + 1) * EMB - k0)
                for dh in range(2):
                    wp[:kk, dh, col, :] = W[dh * 128 : (dh + 1) * 128, k0 : k0 + kk].T
                col += 1

    bp = np.empty((128, 3, 2), np.float32)
    for br, b in enumerate((b1, b2, b3)):
        b = np.asarray(b, np.float32).reshape(DEPTH)
        for dh in range(2):
            bp[:, br, dh] = b[dh * 128 : (dh + 1) * 128]
    return xt, wp, bp


def kernel(input, W1, W2, W3, b1, b2, b3):
    global _built, LAST_RESULT
    from concourse.bass_utils import run_bass_kernel_spmd

    xt, wp, bp = _pack_inputs(input, W1, W2, W3, b1, b2, b3)

    if _built is None:
        _built = _build_bass()
    nc = _built

    in_maps = [
        {"xt": xt[c * BPC : (c + 1) * BPC], "wp": wp, "bp": bp}
        for c in range(NCORES)
    ]
    res = run_bass_kernel_spmd(
        nc, in_maps, core_ids=list(range(NCORES)), trace=TRACE
    )
    LAST_RESULT = res

    out = np.empty((B, 3 * DEPTH), np.float32)
    for c in range(NCORES):
        arr = res.results[c]["out_t"]  # [128, 3, 2, BPC]
        out[c * BPC : (c + 1) * BPC] = arr.transpose(3, 1, 2, 0).reshape(BPC, 768)
    return out
